# revision 33
# baseline (speedup 1.0000x reference)
"""Trainium2 Bass kernel v13 for nn_MultiHeadAttention_53017076301867.

Strategy (8 cores, tensor-parallel over H=16 heads, 2 heads/core):
  - ctx = mask ? global_attn : local_attn per row. The device computes the
    global branch ONLY for the gathered (sorted) mask==1 query positions,
    in blocks of <=512 gathered columns; causal masking of gathered
    queries vs key tiles uses per-key-partition cutoff columns applied
    with one DVE scalar_tensor_tensor (is_ge x mult) per head, bounded to
    the boundary region [ca, ce).
  - Phase 1 warms the PE clock gate with junk matmuls, then runs the
    gathered-Q and K projections jointly, ec-outer, so accumulation
    tracks per-ec DMA chunk arrival (x^T and gathered-x^T are split into
    per-2ec chunk pairs across both HWDGE queues).
  - The V projection, PE-transposes into V2e, and the fp8 local q/k unit
    projections are emitted as tensor *filler* inside the first (largest)
    attention block's score/exp window, with all of the block's et tiles
    buffered (bufs=17) and its AV accumulation drained in one burst after
    V2e completes. ACT exp therefore starts as soon as K/Q are done.
  - Local windowed branch (q<WIN rows, permuted w~=i*16+j order) as in
    v2 minus the mask-blend: rows are projected directly and the host
    picks mask==0 & q<WIN rows; mask==0 & q>=WIN rows are one constant
    row computed on the host.
  - Output projections of each block run as tensor filler inside the
    next block (Proj.step), casts alternate DVE/ACT, and each block's
    rows leave in one batched DMA. Host sums 8 bf16 partials, scatters
    rows, and adds b_proj.
"""

import numpy as np
import ml_dtypes

S, E, H, WIN, D = 2048, 1024, 16, 256, 64
C = S // WIN            # 8 chunks
NCORES = 8
SCALE = 1.0 / (D ** 0.5)  # 0.125
BF = ml_dtypes.bfloat16
F8 = ml_dtypes.float8_e4m3fn

_prog_cache = {}
FLAGS = dict(mask=True, iota=True, warm=True, batched_out=True)


def make_plan(mask):
    m = np.asarray(mask, np.int64).reshape(S)
    gidx = np.where(m == 1)[0]
    n1 = len(gidx)
    widths = []
    r = n1
    while r > 512:
        widths.append(512)
        r -= 512
    widths.append(max(128, -(-r // 128) * 128))
    n1p = sum(widths)
    gpad = np.concatenate([gidx, np.full(n1p - n1, gidx[-1], np.int64)])
    blocks = []
    off = 0
    for w in widths:
        pb = gpad[off:off + w]
        kt = int(pb.max()) // 128 + 1
        tiles = []
        for t in range(kt):
            c_t = int(np.searchsorted(pb, t * 128))
            cut = np.searchsorted(pb, t * 128 + np.arange(128)).astype(np.int64)
            nm = bool((cut > c_t).any())
            ce = min((int(cut.max()) + 3) & ~3, w)
            tiles.append(dict(t=t, ca=c_t & ~3, ce=ce, cut=cut, nm=nm,
                              mcol=-1))
        wr = w if off + w <= n1 else max(4, (min(w, n1 - off) + 3) & ~3)
        blocks.append(dict(off=off, w=w, kt=kt, tiles=tiles, wr=wr))
        off += w
    order = sorted(range(len(blocks)),
                   key=lambda b: (-blocks[b]['w'], -blocks[b]['kt']))
    nt = 0
    roff = 0
    for b in order:
        bl = blocks[b]
        bl['roff'] = roff
        roff += bl['w']
        for ti in bl['tiles']:
            if ti['nm']:
                ti['mcol'] = nt
                nt += 1
    qgw = []
    r = n1p
    while r > 0:
        qgw.append(min(512, r))
        r -= qgw[-1]
    return dict(n1=n1, n1p=n1p, widths=widths, gpad=gpad, blocks=blocks,
                order=order, nt=max(nt, 1), qgw=qgw, nrows=n1p + WIN)


def plan_key(plan):
    parts = [tuple(plan['widths'])]
    for b in plan['order']:
        bl = plan['blocks'][b]
        parts.append((bl['off'], bl['w'], bl['kt'], bl['roff'], bl['wr'],
                      tuple((ti['t'], ti['ca'], ti['nm'], ti['mcol'])
                            for ti in bl['tiles'])))
    return tuple(parts)


def build_program(plan):
    from contextlib import ExitStack
    import concourse.tile as tile
    import concourse.mybir as mybir
    from concourse import bacc
    from concourse.masks import make_identity

    dt = mybir.dt
    f32, bf, f8 = dt.float32, dt.bfloat16, dt.float8e4
    AF = mybir.ActivationFunctionType
    ALU = mybir.AluOpType

    n1p = plan['n1p']
    NT = plan['nt']
    NR = plan['nrows']

    nc = bacc.Bacc("TRN2", target_bir_lowering=False, debug=False)

    xT = nc.dram_tensor("xT", [128, 8, S], bf, kind="ExternalInput").ap()
    xgT = nc.dram_tensor("xgT", [128, 8, n1p], bf, kind="ExternalInput").ap()
    wqkv = nc.dram_tensor("wqkv", [128, 3, 8, 128], bf, kind="ExternalInput").ap()
    xTl = nc.dram_tensor("xTl", [128, 8, 2 * 128], f8, kind="ExternalInput").ap()
    wlqk = nc.dram_tensor("wlqk", [128, 16, 8, 128], f8, kind="ExternalInput").ap()
    lmask = nc.dram_tensor("lmask", [128, 2, WIN], bf, kind="ExternalInput").ap()
    wpr = nc.dram_tensor("wpr", [128, E], bf, kind="ExternalInput").ap()
    v256 = nc.dram_tensor("v256", [128, 2, 2, 65], bf, kind="ExternalInput").ap()
    tailv = nc.dram_tensor("tailv", [1, 2, 65], bf, kind="ExternalInput").ap()
    cuts = nc.dram_tensor("cuts", [128, NT], f32, kind="ExternalInput").ap()
    outp = nc.dram_tensor("outp", [NR, E], bf, kind="ExternalOutput").ap()
    outv = outp.rearrange("(r p) e -> p r e", p=128)

    with tile.TileContext(nc) as tc, ExitStack() as ctx:
        P = ctx.enter_context(tc.tile_pool(name="persist", bufs=1))

        # ---------------- input DMAs (order = per-queue priority) --------
        xgT_sb = P.tile([128, 8, n1p], bf)
        xT_sb = P.tile([128, 8, S], bf)
        wqkv_sb = P.tile([128, 3, 8, 128], bf)
        xTl_sb = P.tile([128, 8, 256], f8)
        wlqk_sb = P.tile([128, 16, 8, 128], f8)
        # per-ec (xgT, xT) pairs split across the two HWDGE queues so the
        # joint ec-outer Q/K accumulation can start as soon as each ec lands
        nc.sync.dma_start(out=wqkv_sb, in_=wqkv)
        nc.scalar.dma_start(out=xTl_sb, in_=xTl)
        for c2 in range(4):
            eng = nc.sync if c2 % 2 == 0 else nc.scalar
            sl = slice(2 * c2, 2 * c2 + 2)
            eng.dma_start(out=xgT_sb[:, sl, :], in_=xgT[:, sl, :])
            eng.dma_start(out=xT_sb[:, sl, :], in_=xT[:, sl, :])
        for q4 in range(4):
            eng = nc.sync if q4 % 2 == 0 else nc.scalar
            eng.dma_start(out=wlqk_sb[:, 4 * q4:4 * q4 + 4, :, :],
                          in_=wlqk[:, 4 * q4:4 * q4 + 4, :, :])
        lmask_sb = P.tile([128, 2, WIN], bf)
        nc.sync.dma_start(out=lmask_sb, in_=lmask)
        v256_sb = P.tile([128, 2, 2, 65], bf)
        nc.scalar.dma_start(out=v256_sb, in_=v256)
        tailv_sb = P.tile([1, 2, 65], bf)
        nc.sync.dma_start(out=tailv_sb, in_=tailv)
        wpr_sb = P.tile([128, E], bf)
        nc.scalar.dma_start(out=wpr_sb, in_=wpr)
        cuts_sb = P.tile([128, NT], f32)
        nc.gpsimd.dma_start(out=cuts_sb, in_=cuts)

        # ---------------- setup ----------------
        warm = P.tile([128, 512], bf)
        nc.vector.memset(warm, 0.125)
        onesrow = P.tile([1, WIN], bf)
        nc.vector.memset(onesrow, 1.0)
        identb128 = P.tile([128, 128], bf)
        make_identity(nc, identb128)
        colidx = P.tile([128, 512], f32)
        nc.gpsimd.iota(colidx, pattern=[[1, 512]], base=0,
                       channel_multiplier=0,
                       allow_small_or_imprecise_dtypes=True)

        QT2g = P.tile([128, n1p], bf)
        KT2 = P.tile([128, S], bf)
        V2e = P.tile([128, 16, 2, 65], bf)
        QP = P.tile([64, 2, 8, 16, 16], bf)
        KP = P.tile([64, 2, 8, 16, 16], bf)
        ctxT = P.tile([128, n1p], bf)
        nc.gpsimd.memset(ctxT, 0.0)
        blocb = P.tile([128, WIN], bf)

        # ------- phase 1a: warm-up, joint ec-outer gathered-Q + K -------
        with tc.tile_pool(name="ps1", bufs=1, space="PSUM") as ps1:
            wps = ps1.tile([128, 512], f32, tag="qg0", bufs=1, name="warmps")
            for _ in range(16):
                nc.tensor.matmul(wps, lhsT=warm[:, 0:128], rhs=warm,
                                 start=True, stop=True, skip_group_check=True)
            qgw = plan['qgw']
            qps = [ps1.tile([128, 512], f32, tag=f"qg{g}", bufs=1,
                            name=f"qgps{g}") for g in range(len(qgw))]
            kps = [ps1.tile([128, 512], f32, tag=f"kv{g}", bufs=1,
                            name=f"kps{g}") for g in range(4)]
            uidx = [0]

            for ec in range(8):
                qoff = 0
                for g, gw in enumerate(qgw):
                    nc.tensor.matmul(qps[g][:, 0:gw],
                                     lhsT=wqkv_sb[:, 0, ec, :],
                                     rhs=xgT_sb[:, ec, qoff:qoff + gw],
                                     start=(ec == 0), stop=(ec == 7),
                                     skip_group_check=True)
                    qoff += gw
                for g in range(4):
                    nc.tensor.matmul(
                        kps[g], lhsT=wqkv_sb[:, 1, ec, :],
                        rhs=xT_sb[:, ec, g * 512:(g + 1) * 512],
                        start=(ec == 0), stop=(ec == 7),
                        skip_group_check=True)
            qoff = 0
            for g, gw in enumerate(qgw):
                nc.vector.tensor_copy(QT2g[:, qoff:qoff + gw],
                                      qps[g][:, 0:gw])
                qoff += gw
            for g in range(4):
                nc.vector.tensor_copy(KT2[:, g * 512:(g + 1) * 512], kps[g])

        # ------- phase 2: V, attention blocks, local, projections -------
        with tc.tile_pool(name="ps3", bufs=2, space="PSUM") as ps3, \
                tc.tile_pool(name="sb3", bufs=4) as sb3:
            # V projection + transpose into V2e — emitted as filler work
            # inside the first attention block's score/exp window
            VT2 = sb3.tile([128, S], bf, tag="vt2", bufs=1)

            def emit_vgroup(g):
                ps = ps3.tile([128, 512], f32, tag="aux", bufs=2, name="vps")
                for ec in range(8):
                    nc.tensor.matmul(
                        ps, lhsT=wqkv_sb[:, 2, ec, :],
                        rhs=xT_sb[:, ec, g * 512:(g + 1) * 512],
                        start=(ec == 0), stop=(ec == 7))
                nc.vector.tensor_copy(VT2[:, g * 512:(g + 1) * 512], ps)

            def emit_vtr(st):
                pv = ps3.tile([128, 128], bf, tag="aux", bufs=2, name="pvps")
                nc.tensor.transpose(pv, VT2[:, st * 128:(st + 1) * 128],
                                    identb128)
                nc.vector.tensor_copy(V2e[:, st, :, 0:64],
                                      pv.rearrange("p (h d) -> p h d", h=2))
                if st == 15:
                    nc.gpsimd.memset(V2e[:, :, :, 64], 1.0)

            fillq = []
            for g in range(4):
                fillq.append(lambda g=g: emit_vgroup(g))
            for s4 in range(4):
                def vtr4(s4=s4):
                    for st in range(4 * s4, 4 * s4 + 4):
                        emit_vtr(st)
                fillq.append(vtr4)

            def fill_one():
                if fillq:
                    fillq.pop(0)()
                else:
                    emit_units(1)

            # local q/k units, emitted as filler inside the first block

            wlqk_dr = wlqk_sb.rearrange("p i (g j) m -> p i g j m", j=2)
            xTl_dr = xTl_sb.rearrange("p (g j) s -> p g j s", j=2)

            def emit_units(n):
                while uidx[0] < 16 and n > 0:
                    i = uidx[0]
                    uidx[0] += 1
                    n -= 1
                    ps = ps3.tile([128, 256], f32, tag="aux", bufs=2,
                                  name="ups")
                    for ec in range(8):
                        nc.tensor.matmul(ps, lhsT=wlqk_sb[:, i, ec, :],
                                         rhs=xTl_sb[:, ec, :],
                                         start=(ec == 0), stop=(ec == 7))
                    qsrc = ps[0:64, :].rearrange("d (h u j) -> d h u j",
                                                 h=2, u=8)
                    ksrc = ps[64:128, :].rearrange("d (h u j) -> d h u j",
                                                   h=2, u=8)
                    nc.vector.tensor_copy(QP[:, :, :, i, :], qsrc)
                    nc.vector.tensor_copy(KP[:, :, :, i, :], ksrc)

            mcnt = [0]

            def emit_mask(et, sl_et, iosl, cutcol):
                mcnt[0] += 1
                nc.vector.scalar_tensor_tensor(
                    out=et[sl_et], in0=colidx[:, iosl], scalar=cutcol,
                    in1=et[sl_et], op0=ALU.is_ge, op1=ALU.mult)

            def do_block(bl, filler=None, drain_at=2, etbufs=17,
                         drain_gate=None):
                w, off, kt, wr = bl['w'], bl['off'], bl['kt'], bl['wr']
                pack = 512 // w
                tiles = bl['tiles']
                ngrp = -(-len(tiles) // pack)
                gpss = ps3.tile([65, 2, 512], f32, tag="g01", bufs=1,
                                name="gctxps")
                pend = []
                for gi in range(2 * ngrp + 2):
                    if gi < ngrp:
                        grp = tiles[gi * pack:(gi + 1) * pack]
                        ca0 = grp[0]['ca'] if pack == 1 else 0
                        sps = ps3.tile([128, 2, 512], f32, tag="sT", bufs=2,
                                       name="sTps")
                        for qi, ti in enumerate(grp):
                            t, ca = ti['t'], ti['ca']
                            for hh in range(2):
                                hs = slice(hh * 64, hh * 64 + 64)
                                nc.tensor.matmul(
                                    sps[:, hh, qi * w + ca:qi * w + wr],
                                    lhsT=KT2[hs, t * 128:(t + 1) * 128],
                                    rhs=QT2g[hs, off + ca:off + wr],
                                    start=(qi == 0), stop=(qi == len(grp) - 1),
                                    skip_group_check=True)
                        et = sb3.tile([128, 2, 512], bf, tag="expT",
                                      bufs=etbufs, name="etT")
                        if pack == 1:
                            nc.scalar.activation(et[:, :, ca0:wr],
                                                 sps[:, :, ca0:wr], AF.Exp,
                                                 scale=SCALE)
                        else:
                            etv = et.rearrange("p h (g q) -> p h g q", q=w)
                            spsv = sps.rearrange("p h (g q) -> p h g q", q=w)
                            nc.scalar.activation(
                                etv[:, :, 0:len(grp), ca0:wr],
                                spsv[:, :, 0:len(grp), ca0:wr], AF.Exp,
                                scale=SCALE)
                        for qi, ti in enumerate(grp):
                            if ti['nm'] and FLAGS['mask']:
                                ca = ti['ca']
                                ce = min(ti['ce'], wr)
                                cutcol = cuts_sb[:, ti['mcol']:ti['mcol'] + 1]
                                for hh in range(2):
                                    emit_mask(
                                        et, (slice(None), hh,
                                             slice(qi * w + ca, qi * w + ce)),
                                        slice(ca, ce), cutcol)
                        pend.append((grp, et))
                        if filler is not None:
                            filler()
                    ready = drain_gate is None or drain_gate()
                    if (ready and len(pend) > drain_at) or (gi >= ngrp
                                                           and pend):
                        pgrp, pet = pend.pop(0)
                        for qi, ti in enumerate(pgrp):
                            t, ca = ti['t'], ti['ca']
                            for hh in range(2):
                                nc.tensor.matmul(
                                    gpss[:, hh, ca:wr],
                                    lhsT=V2e[:, t, hh, :],
                                    rhs=pet[:, hh, qi * w + ca:qi * w + wr],
                                    start=(t == 0), stop=(t == kt - 1),
                                    skip_group_check=True)
                # blend: ctxT = gpss[0:64] / gpss[64]
                zsr = sb3.tile([1, 2, wr], f32, tag=f"zsr{w}", bufs=2)
                nc.vector.tensor_copy(zsr, gpss[64:65, :, 0:wr])
                zrow = sb3.tile([1, 2, wr], f32, tag=f"zrow{w}", bufs=2)
                nc.vector.reciprocal_approx_fast(zrow, zsr)
                rbs = sb3.tile([64, 2, wr], f32, tag=f"rbs{w}", bufs=2)
                nc.gpsimd.partition_broadcast(rbs, zrow)
                for hh in range(2):
                    hs = slice(hh * 64, hh * 64 + 64)
                    nc.vector.tensor_mul(ctxT[hs, off:off + wr],
                                         gpss[0:64, hh, 0:wr], rbs[:, hh, :])

            pcnt = [0]

            class Proj:
                def __init__(self, colbase, roff, nqt, src, bname):
                    self.colbase, self.roff, self.nqt = colbase, roff, nqt
                    self.src, self.bname = src, bname
                    self.idx = 0
                    self.stg = sb3.tile([128, nqt, E], bf,
                                        tag=f"stg{bname}", bufs=1,
                                        name=f"stg{bname}")

                def step(self, n=1):
                    while self.idx < 2 * self.nqt and n > 0:
                        j, half = divmod(self.idx, 2)
                        self.idx += 1
                        n -= 1
                        pp = ps3.tile([128, 512], f32, tag="aux", bufs=2,
                                      name="ppps")
                        nc.tensor.matmul(
                            pp, lhsT=self.src[:, self.colbase + j * 128:
                                              self.colbase + (j + 1) * 128],
                            rhs=wpr_sb[:, half * 512:(half + 1) * 512],
                            start=True, stop=True)
                        dst = self.stg[:, j, half * 512:(half + 1) * 512]
                        pcnt[0] += 1
                        if pcnt[0] % 2:
                            nc.vector.tensor_copy(dst, pp)
                        else:
                            nc.scalar.copy(dst, pp)

                def finish(self):
                    self.step(2 * self.nqt)
                    if FLAGS['batched_out']:
                        nc.sync.dma_start(
                            out=outv[:, self.roff // 128:
                                     self.roff // 128 + self.nqt, :],
                            in_=self.stg)
                    else:
                        for j in range(self.nqt):
                            nc.sync.dma_start(
                                out=outp[self.roff + j * 128:
                                         self.roff + (j + 1) * 128, :],
                                in_=self.stg[:, j, :])

            def emit_proj(colbase, roff, nqt, src, bname, cast_eng=None):
                Proj(colbase, roff, nqt, src, bname).finish()

            # ---- blocks in order; local chain after the second block ----
            order = plan['order']
            blocks = plan['blocks']

            def mkproj(b):
                bl = blocks[b]
                return Proj(bl['off'], bl['roff'], bl['w'] // 128, ctxT,
                            str(b))

            do_block(blocks[order[0]], filler=fill_one, drain_at=2,
                     drain_gate=lambda: not fillq)
            while fillq:
                fillq.pop(0)()
            emit_units(16)
            prev = order[0]
            rest = list(order[1:])
            pj = [None]
            if rest:
                b = rest.pop(0)
                pjx = mkproj(prev)
                do_block(blocks[b], filler=lambda: pjx.step(2))
                pjx.finish()
                prev = b
            pj[0] = mkproj(prev)
            # ---- local windowed attention ----
            slocs = {}
            ets = {}
            for k2 in range(2):
                sps = ps3.tile([128, 2, 512], f32, tag="sT", bufs=2,
                               name=f"slocps{k2}")
                for hh in range(2):
                    for u in range(8):
                        nc.tensor.matmul(
                            sps[:, hh, 0:WIN],
                            lhsT=KP[:, hh, u, k2 * 8:(k2 + 1) * 8, :],
                            rhs=QP[:, hh, u, :, :],
                            start=(u == 0), stop=(u == 7),
                            skip_group_check=True)
                slocs[k2] = sps
                pj[0].step(2)
            for k2 in range(2):
                et = sb3.tile([128, 2, WIN], bf, tag="eloc", bufs=2,
                              name=f"eloc{k2}")
                nc.scalar.activation(et, slocs[k2][:, :, 0:WIN], AF.Exp,
                                     scale=SCALE / C)
                for hh in range(2):
                    nc.vector.tensor_mul(et[:, hh, :], et[:, hh, :],
                                         lmask_sb[:, k2, :])
                    ets[(k2, hh)] = et[:, hh, :]
            for hh in range(2):
                ploc = ps3.tile([65, WIN], f32, tag="aux", bufs=2,
                                name=f"plocps{hh}")
                for k2 in range(2):
                    nc.tensor.matmul(ploc, lhsT=v256_sb[:, hh, k2, :],
                                     rhs=ets[(k2, hh)], start=(k2 == 0),
                                     stop=False, skip_group_check=True)
                nc.tensor.matmul(ploc, lhsT=tailv_sb[:, hh, :],
                                 rhs=onesrow, start=False, stop=True,
                                 skip_group_check=True)
                zsl = sb3.tile([1, WIN], f32, tag="zsl", bufs=2)
                nc.vector.tensor_copy(zsl, ploc[64:65, :])
                zl = sb3.tile([1, WIN], f32, tag="zl", bufs=2)
                nc.vector.reciprocal_approx_fast(zl, zsl)
                rbls = sb3.tile([64, WIN], f32, tag="rbls", bufs=2)
                nc.gpsimd.partition_broadcast(rbls, zl)
                nc.vector.tensor_mul(blocb[hh * 64:(hh + 1) * 64, :],
                                     ploc[0:64, :], rbls)
            pj[0].step(2)
            emit_proj(0, n1p, 2, blocb, "loc")
            for b in rest:
                pjc = pj[0]
                do_block(blocks[b], filler=lambda: pjc.step(3))
                pjc.finish()
                prev = b
                pj[0] = mkproj(prev)
            pj[0].finish()
    nc.compile()
    return nc


def prep_inputs(x, global_attention_mask, W_local_query, W_local_key,
                W_local_value, W_query, W_key, W_value, W_proj, plan=None):
    """Host-side sharding/layout prep. Returns (plan, per-core input dicts,
    const_ctx_row[E])."""
    if plan is None:
        plan = make_plan(global_attention_mask)

    def b(a):
        return np.ascontiguousarray(np.asarray(a, np.float32)).astype(BF)

    x2 = np.asarray(x, np.float32).reshape(S, E)
    xT_np = np.ascontiguousarray(
        x2.T.reshape(8, 128, S).transpose(1, 0, 2)).astype(BF)
    xg = x2[plan['gpad']]
    xgT_np = np.ascontiguousarray(
        xg.T.reshape(8, 128, plan['n1p']).transpose(1, 0, 2)).astype(BF)

    Wq = np.asarray(W_query, np.float32)
    Wk = np.asarray(W_key, np.float32)
    Wv = np.asarray(W_value, np.float32)
    Wp = np.asarray(W_proj, np.float32)
    Wlv = np.asarray(W_local_value, np.float32)

    # local unit weights: per-i interleave [q_i | k_i], [p, i, c, v]
    Wlq = np.asarray(W_local_query, np.float32).reshape(E, 16, 64)
    Wlk = np.asarray(W_local_key, np.float32).reshape(E, 16, 64)
    wlqk_e = np.concatenate([Wlq, Wlk], axis=2)               # [E, 16, 128]
    wlqk_np = np.ascontiguousarray(
        wlqk_e.reshape(8, 128, 16, 128).transpose(1, 2, 0, 3)).astype(F8)
    wt = np.arange(WIN)
    w_of = (wt % 16) * 16 + wt // 16
    lmask_np = np.ascontiguousarray(
        (w_of.reshape(2, 128)[:, :, None] <= w_of[None, None, :])
        .astype(np.float32).transpose(1, 0, 2)).astype(BF)    # [128, 2, WIN]

    # host-computed local-value summaries
    rows16 = (np.arange(H)[:, None] * 128 + np.arange(16)[None, :]).ravel()
    vl16 = (x2[rows16] @ Wlv).reshape(H, 16, E)
    xsumA = x2.reshape(H, 128, E).sum(axis=1)
    colsumA = xsumA @ Wlv
    colsum16 = vl16.sum(axis=1)
    vbarH = colsumA.reshape(H, 16, 64).sum(axis=1)            # [H, 64]
    tailH = (colsumA - colsum16).reshape(H, 16, 64).sum(axis=1)
    const_row = (vbarH.reshape(E) / S) @ Wp                   # [E]

    # causal cutoff columns for masked tiles
    NT = plan['nt']
    cuts_np = np.zeros((128, NT), np.float32)
    for bidx in plan['order']:
        bl = plan['blocks'][bidx]
        for ti in bl['tiles']:
            if ti['nm']:
                cuts_np[:, ti['mcol']] = ti['cut'] - 0.5

    in_maps = []
    for i in range(NCORES):
        cs = slice(i * 128, (i + 1) * 128)
        wqkv_np = np.stack([
            np.ascontiguousarray(
                W[:, cs].reshape(8, 128, 128).transpose(1, 0, 2))
            for W in (Wq, Wk, Wv)], axis=1).astype(BF)        # [128, 3, 8, 128]
        xTl_np = np.ascontiguousarray(
            x2.T[:, i * 256:(i + 1) * 256]
            .reshape(8, 128, 256).transpose(1, 0, 2)).astype(F8)
        v256_np = np.zeros((128, 2, 2, 65), np.float32)
        tail_np = np.zeros((1, 2, 65), np.float32)
        for hh in range(2):
            hg = 2 * i + hh
            for k2 in range(2):
                wt_ = k2 * 128 + np.arange(128)
                k_true = 16 * (wt_ % 16) + wt_ // 16
                r = k_true // 16
                cpos = k_true % 16
                v256_np[:, hh, k2, 0:64] = vl16[
                    hg, r][np.arange(128)[:, None],
                           (cpos * 64)[:, None] + np.arange(64)[None, :]]
            v256_np[:, hh, :, 64] = 1.0
            tail_np[0, hh, 0:64] = tailH[hg]
            tail_np[0, hh, 64] = S - WIN
        in_maps.append({
            "xT": xT_np,
            "xgT": xgT_np,
            "wqkv": wqkv_np,
            "xTl": xTl_np,
            "wlqk": wlqk_np,
            "lmask": lmask_np,
            "wpr": b(Wp[cs, :]),
            "v256": v256_np.astype(BF),
            "tailv": tail_np.astype(BF),
            "cuts": cuts_np,
        })
    return plan, in_maps, const_row


def assemble(plan, partials, const_row, b_proj, global_attention_mask):
    m = np.asarray(global_attention_mask, np.int64).reshape(S)
    bp = np.asarray(b_proj, np.float32)
    acc = np.zeros((plan['nrows'], E), np.float32)
    for r in partials:
        acc += np.asarray(r["outp"], np.float32)
    out = np.zeros((S, E), np.float32)
    # gathered rows: block b's cols [off, off+w) live at outp rows
    # [roff, roff+w)
    gpad = plan['gpad']
    n1 = plan['n1']
    grows = np.empty(plan['n1p'], np.int64)
    for bidx in plan['order']:
        bl = plan['blocks'][bidx]
        grows[bl['off']:bl['off'] + bl['w']] = np.arange(
            bl['roff'], bl['roff'] + bl['w'])
    out[gpad[:n1]] = acc[grows[:n1]]
    m0 = np.where(m == 0)[0]
    out[m0[m0 >= WIN]] = const_row
    loc_rows = m0[m0 < WIN]
    wperm = (loc_rows % 16) * 16 + loc_rows // 16
    out[loc_rows] = acc[plan['n1p'] + wperm]
    out += bp[None, :]
    return out


def kernel(x, global_attention_mask, W_local_query, W_local_key, W_local_value,
           W_query, W_key, W_value, W_proj, b_proj):
    from concourse.bass_utils import run_bass_kernel_spmd

    plan = make_plan(global_attention_mask)
    key = plan_key(plan)
    if key not in _prog_cache:
        _prog_cache[key] = build_program(plan)
    nc = _prog_cache[key]

    plan, in_maps, const_row = prep_inputs(
        x, global_attention_mask, W_local_query, W_local_key, W_local_value,
        W_query, W_key, W_value, W_proj, plan=plan)
    res = run_bass_kernel_spmd(nc, in_maps, core_ids=list(range(NCORES)))
    out = assemble(plan, res.results, const_row, b_proj,
                   global_attention_mask)
    return out[None].astype(np.float32)


# revision 34
# speedup vs baseline: 1.0372x; 1.0372x over previous
"""Trainium2 Bass kernel v13 for nn_MultiHeadAttention_53017076301867.

Strategy (8 cores, tensor-parallel over H=16 heads, 2 heads/core):
  - ctx = mask ? global_attn : local_attn per row. The device computes the
    global branch ONLY for the gathered (sorted) mask==1 query positions,
    in blocks of <=512 gathered columns; causal masking of gathered
    queries vs key tiles uses per-key-partition cutoff columns applied
    with one DVE scalar_tensor_tensor (is_ge x mult) per head, bounded to
    the boundary region [ca, ce).
  - Phase 1 warms the PE clock gate with junk matmuls, then runs the
    gathered-Q and K projections jointly, ec-outer, so accumulation
    tracks per-ec DMA chunk arrival (x^T and gathered-x^T are split into
    per-2ec chunk pairs across both HWDGE queues).
  - The V projection, PE-transposes into V2e, and the fp8 local q/k unit
    projections are emitted as tensor *filler* inside the first (largest)
    attention block's score/exp window, with all of the block's et tiles
    buffered (bufs=17) and its AV accumulation drained in one burst after
    V2e completes. ACT exp therefore starts as soon as K/Q are done.
  - Local windowed branch (q<WIN rows, permuted w~=i*16+j order) as in
    v2 minus the mask-blend: rows are projected directly and the host
    picks mask==0 & q<WIN rows; mask==0 & q>=WIN rows are one constant
    row computed on the host.
  - Output projections of each block run as tensor filler inside the
    next block (Proj.step), casts alternate DVE/ACT, and each block's
    rows leave in one batched DMA. Host sums 8 bf16 partials, scatters
    rows, and adds b_proj.
"""

import numpy as np
import ml_dtypes

S, E, H, WIN, D = 2048, 1024, 16, 256, 64
C = S // WIN            # 8 chunks
NCORES = 8
SCALE = 1.0 / (D ** 0.5)  # 0.125
BF = ml_dtypes.bfloat16
F8 = ml_dtypes.float8_e4m3fn

_prog_cache = {}
FLAGS = dict(mask=True, iota=True, warm=True, batched_out=True)


def make_plan(mask):
    m = np.asarray(mask, np.int64).reshape(S)
    gidx = np.where(m == 1)[0]
    n1 = len(gidx)
    widths = []
    r = n1
    while r > 512:
        widths.append(512)
        r -= 512
    widths.append(max(128, -(-r // 128) * 128))
    n1p = sum(widths)
    gpad = np.concatenate([gidx, np.full(n1p - n1, gidx[-1], np.int64)])
    blocks = []
    off = 0
    for w in widths:
        pb = gpad[off:off + w]
        kt = int(pb.max()) // 128 + 1
        tiles = []
        for t in range(kt):
            c_t = int(np.searchsorted(pb, t * 128))
            cut = np.searchsorted(pb, t * 128 + np.arange(128)).astype(np.int64)
            nm = bool((cut > c_t).any())
            ce = min((int(cut.max()) + 3) & ~3, w)
            tiles.append(dict(t=t, ca=c_t & ~3, ce=ce, cut=cut, nm=nm,
                              mcol=-1))
        wr = w if off + w <= n1 else max(4, (min(w, n1 - off) + 3) & ~3)
        blocks.append(dict(off=off, w=w, kt=kt, tiles=tiles, wr=wr))
        off += w
    order = sorted(range(len(blocks)),
                   key=lambda b: (-blocks[b]['w'], -blocks[b]['kt']))
    nt = 0
    roff = 0
    for b in order:
        bl = blocks[b]
        bl['roff'] = roff
        roff += bl['w']
        for ti in bl['tiles']:
            if ti['nm']:
                ti['mcol'] = nt
                nt += 1
    qgw = []
    r = n1p
    while r > 0:
        qgw.append(min(512, r))
        r -= qgw[-1]
    return dict(n1=n1, n1p=n1p, widths=widths, gpad=gpad, blocks=blocks,
                order=order, nt=max(nt, 1), qgw=qgw, nrows=n1p + WIN)


def plan_key(plan):
    parts = [tuple(plan['widths'])]
    for b in plan['order']:
        bl = plan['blocks'][b]
        parts.append((bl['off'], bl['w'], bl['kt'], bl['roff'], bl['wr'],
                      tuple((ti['t'], ti['ca'], ti['nm'], ti['mcol'])
                            for ti in bl['tiles'])))
    return tuple(parts)


def build_program(plan):
    from contextlib import ExitStack
    import concourse.tile as tile
    import concourse.mybir as mybir
    from concourse import bacc
    from concourse.masks import make_identity

    dt = mybir.dt
    f32, bf, f8 = dt.float32, dt.bfloat16, dt.float8e4
    AF = mybir.ActivationFunctionType
    ALU = mybir.AluOpType

    n1p = plan['n1p']
    NT = plan['nt']
    NR = plan['nrows']

    nc = bacc.Bacc("TRN2", target_bir_lowering=False, debug=False)

    xT = nc.dram_tensor("xT", [128, 8, S], bf, kind="ExternalInput").ap()
    xgT = nc.dram_tensor("xgT", [128, 8, n1p], bf, kind="ExternalInput").ap()
    wqkv = nc.dram_tensor("wqkv", [128, 3, 8, 128], bf, kind="ExternalInput").ap()
    xTl = nc.dram_tensor("xTl", [128, 8, 2 * 128], f8, kind="ExternalInput").ap()
    wlqk = nc.dram_tensor("wlqk", [128, 16, 8, 128], f8, kind="ExternalInput").ap()
    lmask = nc.dram_tensor("lmask", [128, 2, WIN], bf, kind="ExternalInput").ap()
    wpr = nc.dram_tensor("wpr", [128, E], bf, kind="ExternalInput").ap()
    v256 = nc.dram_tensor("v256", [128, 2, 2, 65], bf, kind="ExternalInput").ap()
    tailv = nc.dram_tensor("tailv", [1, 2, 65], bf, kind="ExternalInput").ap()
    cuts = nc.dram_tensor("cuts", [128, NT], f32, kind="ExternalInput").ap()
    outp = nc.dram_tensor("outp", [NR, E], bf, kind="ExternalOutput").ap()
    outv = outp.rearrange("(r p) e -> p r e", p=128)

    with tile.TileContext(nc) as tc, ExitStack() as ctx:
        P = ctx.enter_context(tc.tile_pool(name="persist", bufs=1))

        # ---------------- input DMAs (order = per-queue priority) --------
        xgT_sb = P.tile([128, 8, n1p], bf)
        xT_sb = P.tile([128, 8, S], bf)
        wqkv_sb = P.tile([128, 3, 8, 128], bf)
        xTl_sb = P.tile([128, 8, 256], f8)
        wlqk_sb = P.tile([128, 16, 8, 128], f8)
        # per-ec (xgT, xT) pairs split across the two HWDGE queues so the
        # joint ec-outer Q/K accumulation can start as soon as each ec lands
        nc.sync.dma_start(out=wqkv_sb, in_=wqkv)
        nc.scalar.dma_start(out=xTl_sb, in_=xTl)
        for c2 in range(4):
            eng = nc.sync if c2 % 2 == 0 else nc.scalar
            sl = slice(2 * c2, 2 * c2 + 2)
            eng.dma_start(out=xgT_sb[:, sl, :], in_=xgT[:, sl, :])
            eng.dma_start(out=xT_sb[:, sl, :], in_=xT[:, sl, :])
        for q4 in range(4):
            eng = nc.sync if q4 % 2 == 0 else nc.scalar
            eng.dma_start(out=wlqk_sb[:, 4 * q4:4 * q4 + 4, :, :],
                          in_=wlqk[:, 4 * q4:4 * q4 + 4, :, :])
        lmask_sb = P.tile([128, 2, WIN], bf)
        nc.sync.dma_start(out=lmask_sb, in_=lmask)
        v256_sb = P.tile([128, 2, 2, 65], bf)
        nc.scalar.dma_start(out=v256_sb, in_=v256)
        tailv_sb = P.tile([1, 2, 65], bf)
        nc.sync.dma_start(out=tailv_sb, in_=tailv)
        wpr_sb = P.tile([128, E], bf)
        nc.scalar.dma_start(out=wpr_sb, in_=wpr)
        cuts_sb = P.tile([128, NT], f32)
        nc.gpsimd.dma_start(out=cuts_sb, in_=cuts)

        # ---------------- setup ----------------
        warm = P.tile([128, 512], bf)
        nc.vector.memset(warm, 0.125)
        onesrow = P.tile([1, WIN], bf)
        nc.vector.memset(onesrow, 1.0)
        identb128 = P.tile([128, 128], bf)
        make_identity(nc, identb128)
        colidx = P.tile([128, 512], f32)
        nc.gpsimd.iota(colidx, pattern=[[1, 512]], base=0,
                       channel_multiplier=0,
                       allow_small_or_imprecise_dtypes=True)

        QT2g = P.tile([128, n1p], bf)
        KT2 = P.tile([128, S], bf)
        V2e = P.tile([128, 16, 2, 65], bf)
        QP = P.tile([64, 2, 8, 16, 16], bf)
        KP = P.tile([64, 2, 8, 16, 16], bf)
        ctxT = P.tile([128, n1p], bf)
        nc.gpsimd.memset(ctxT, 0.0)
        blocb = P.tile([128, WIN], bf)

        # ------- phase 1a: warm-up, joint ec-outer gathered-Q + K -------
        with tc.tile_pool(name="ps1", bufs=1, space="PSUM") as ps1:
            wps = ps1.tile([128, 512], f32, tag="qg0", bufs=1, name="warmps")
            for _ in range(16):
                nc.tensor.matmul(wps, lhsT=warm[:, 0:128], rhs=warm,
                                 start=True, stop=True, skip_group_check=True)
            qgw = plan['qgw']
            qps = [ps1.tile([128, 512], f32, tag=f"qg{g}", bufs=1,
                            name=f"qgps{g}") for g in range(len(qgw))]
            kps = [ps1.tile([128, 512], f32, tag=f"kv{g}", bufs=1,
                            name=f"kps{g}") for g in range(4)]
            uidx = [0]

            for ec in range(8):
                qoff = 0
                for g, gw in enumerate(qgw):
                    nc.tensor.matmul(qps[g][:, 0:gw],
                                     lhsT=wqkv_sb[:, 0, ec, :],
                                     rhs=xgT_sb[:, ec, qoff:qoff + gw],
                                     start=(ec == 0), stop=(ec == 7),
                                     skip_group_check=True)
                    qoff += gw
                for g in range(4):
                    nc.tensor.matmul(
                        kps[g], lhsT=wqkv_sb[:, 1, ec, :],
                        rhs=xT_sb[:, ec, g * 512:(g + 1) * 512],
                        start=(ec == 0), stop=(ec == 7),
                        skip_group_check=True)
            qoff = 0
            for g, gw in enumerate(qgw):
                nc.vector.tensor_copy(QT2g[:, qoff:qoff + gw],
                                      qps[g][:, 0:gw])
                qoff += gw
            for g in range(4):
                nc.vector.tensor_copy(KT2[:, g * 512:(g + 1) * 512], kps[g])

        # ------- phase 2: V, attention blocks, local, projections -------
        with tc.tile_pool(name="ps3", bufs=2, space="PSUM") as ps3, \
                tc.tile_pool(name="sb3", bufs=4) as sb3:
            # V projection + transpose into V2e — emitted as filler work
            # inside the first attention block's score/exp window
            VT2 = sb3.tile([128, S], bf, tag="vt2", bufs=1)

            def emit_vgroup(g):
                ps = ps3.tile([128, 512], f32, tag="aux", bufs=2, name="vps")
                for ec in range(8):
                    nc.tensor.matmul(
                        ps, lhsT=wqkv_sb[:, 2, ec, :],
                        rhs=xT_sb[:, ec, g * 512:(g + 1) * 512],
                        start=(ec == 0), stop=(ec == 7))
                nc.vector.tensor_copy(VT2[:, g * 512:(g + 1) * 512], ps)

            def emit_vtr(st):
                pv = ps3.tile([128, 128], bf, tag="aux", bufs=2, name="pvps")
                nc.tensor.transpose(pv, VT2[:, st * 128:(st + 1) * 128],
                                    identb128)
                nc.vector.tensor_copy(V2e[:, st, :, 0:64],
                                      pv.rearrange("p (h d) -> p h d", h=2))
                if st == 15:
                    nc.gpsimd.memset(V2e[:, :, :, 64], 1.0)

            fillq = []
            for g in range(4):
                fillq.append(lambda g=g: emit_vgroup(g))
            for s4 in range(4):
                def vtr4(s4=s4):
                    for st in range(4 * s4, 4 * s4 + 4):
                        emit_vtr(st)
                fillq.append(vtr4)

            def fill_one():
                if fillq:
                    fillq.pop(0)()
                else:
                    emit_units(1)

            # local q/k units, emitted as filler inside the first block

            wlqk_dr = wlqk_sb.rearrange("p i (g j) m -> p i g j m", j=2)
            xTl_dr = xTl_sb.rearrange("p (g j) s -> p g j s", j=2)

            def emit_units(n):
                while uidx[0] < 16 and n > 0:
                    i = uidx[0]
                    uidx[0] += 1
                    n -= 1
                    ps = ps3.tile([128, 256], f32, tag="aux", bufs=2,
                                  name="ups")
                    for ec in range(8):
                        nc.tensor.matmul(ps, lhsT=wlqk_sb[:, i, ec, :],
                                         rhs=xTl_sb[:, ec, :],
                                         start=(ec == 0), stop=(ec == 7))
                    qsrc = ps[0:64, :].rearrange("d (h u j) -> d h u j",
                                                 h=2, u=8)
                    ksrc = ps[64:128, :].rearrange("d (h u j) -> d h u j",
                                                   h=2, u=8)
                    nc.vector.tensor_copy(QP[:, :, :, i, :], qsrc)
                    nc.vector.tensor_copy(KP[:, :, :, i, :], ksrc)

            mcnt = [0]

            def emit_mask(et, sl_et, iosl, cutcol):
                mcnt[0] += 1
                nc.vector.scalar_tensor_tensor(
                    out=et[sl_et], in0=colidx[:, iosl], scalar=cutcol,
                    in1=et[sl_et], op0=ALU.is_ge, op1=ALU.mult)

            def do_block(bl, filler=None, drain_at=2, etbufs=17,
                         drain_gate=None):
                w, off, kt, wr = bl['w'], bl['off'], bl['kt'], bl['wr']
                pack = 512 // w
                tiles = bl['tiles']
                ngrp = -(-len(tiles) // pack)
                gpss = ps3.tile([65, 2, 512], f32, tag="g01", bufs=1,
                                name="gctxps")
                pend = []
                for gi in range(2 * ngrp + 2):
                    if gi < ngrp:
                        grp = tiles[gi * pack:(gi + 1) * pack]
                        ca0 = grp[0]['ca'] if pack == 1 else 0
                        sps = ps3.tile([128, 2, 512], f32, tag="sT", bufs=2,
                                       name="sTps")
                        for qi, ti in enumerate(grp):
                            t, ca = ti['t'], ti['ca']
                            for hh in range(2):
                                hs = slice(hh * 64, hh * 64 + 64)
                                nc.tensor.matmul(
                                    sps[:, hh, qi * w + ca:qi * w + wr],
                                    lhsT=KT2[hs, t * 128:(t + 1) * 128],
                                    rhs=QT2g[hs, off + ca:off + wr],
                                    start=(qi == 0), stop=(qi == len(grp) - 1),
                                    skip_group_check=True)
                        et = sb3.tile([128, 2, 512], bf, tag="expT",
                                      bufs=etbufs, name="etT")
                        if pack == 1:
                            nc.scalar.activation(et[:, :, ca0:wr],
                                                 sps[:, :, ca0:wr], AF.Exp,
                                                 scale=SCALE)
                        else:
                            etv = et.rearrange("p h (g q) -> p h g q", q=w)
                            spsv = sps.rearrange("p h (g q) -> p h g q", q=w)
                            nc.scalar.activation(
                                etv[:, :, 0:len(grp), ca0:wr],
                                spsv[:, :, 0:len(grp), ca0:wr], AF.Exp,
                                scale=SCALE)
                        for qi, ti in enumerate(grp):
                            if ti['nm'] and FLAGS['mask']:
                                ca = ti['ca']
                                ce = min(ti['ce'], wr)
                                cutcol = cuts_sb[:, ti['mcol']:ti['mcol'] + 1]
                                for hh in range(2):
                                    emit_mask(
                                        et, (slice(None), hh,
                                             slice(qi * w + ca, qi * w + ce)),
                                        slice(ca, ce), cutcol)
                        pend.append((grp, et))
                        if filler is not None:
                            filler()
                    ready = drain_gate is None or drain_gate()
                    if (ready and len(pend) > drain_at) or (gi >= ngrp
                                                           and pend):
                        pgrp, pet = pend.pop(0)
                        for qi, ti in enumerate(pgrp):
                            t, ca = ti['t'], ti['ca']
                            for hh in range(2):
                                nc.tensor.matmul(
                                    gpss[:, hh, ca:wr],
                                    lhsT=V2e[:, t, hh, :],
                                    rhs=pet[:, hh, qi * w + ca:qi * w + wr],
                                    start=(t == 0), stop=(t == kt - 1),
                                    skip_group_check=True)
                # blend: ctxT = gpss[0:64] / gpss[64]
                zsr = sb3.tile([1, 2, wr], f32, tag=f"zsr{w}", bufs=2)
                nc.vector.tensor_copy(zsr, gpss[64:65, :, 0:wr])
                zrow = sb3.tile([1, 2, wr], f32, tag=f"zrow{w}", bufs=2)
                nc.vector.reciprocal_approx_fast(zrow, zsr)
                rbs = sb3.tile([64, 2, wr], f32, tag=f"rbs{w}", bufs=2)
                nc.gpsimd.partition_broadcast(rbs, zrow)
                for hh in range(2):
                    hs = slice(hh * 64, hh * 64 + 64)
                    nc.vector.tensor_mul(ctxT[hs, off:off + wr],
                                         gpss[0:64, hh, 0:wr], rbs[:, hh, :])

            pcnt = [0]

            class Proj:
                def __init__(self, colbase, roff, nqt, src, bname):
                    self.colbase, self.roff, self.nqt = colbase, roff, nqt
                    self.src, self.bname = src, bname
                    self.idx = 0
                    self.stg = sb3.tile([128, nqt, E], bf,
                                        tag=f"stg{bname}", bufs=1,
                                        name=f"stg{bname}")

                def step(self, n=1):
                    while self.idx < 2 * self.nqt and n > 0:
                        j, half = divmod(self.idx, 2)
                        self.idx += 1
                        n -= 1
                        pp = ps3.tile([128, 512], f32, tag="aux", bufs=2,
                                      name="ppps")
                        nc.tensor.matmul(
                            pp, lhsT=self.src[:, self.colbase + j * 128:
                                              self.colbase + (j + 1) * 128],
                            rhs=wpr_sb[:, half * 512:(half + 1) * 512],
                            start=True, stop=True)
                        dst = self.stg[:, j, half * 512:(half + 1) * 512]
                        pcnt[0] += 1
                        if pcnt[0] % 2:
                            nc.vector.tensor_copy(dst, pp)
                        else:
                            nc.scalar.copy(dst, pp)

                def finish(self):
                    self.step(2 * self.nqt)
                    if FLAGS['batched_out']:
                        nc.sync.dma_start(
                            out=outv[:, self.roff // 128:
                                     self.roff // 128 + self.nqt, :],
                            in_=self.stg)
                    else:
                        for j in range(self.nqt):
                            nc.sync.dma_start(
                                out=outp[self.roff + j * 128:
                                         self.roff + (j + 1) * 128, :],
                                in_=self.stg[:, j, :])

            def emit_proj(colbase, roff, nqt, src, bname, cast_eng=None):
                Proj(colbase, roff, nqt, src, bname).finish()

            # ---- blocks in order; local chain after the second block ----
            order = plan['order']
            blocks = plan['blocks']

            def mkproj(b):
                bl = blocks[b]
                return Proj(bl['off'], bl['roff'], bl['w'] // 128, ctxT,
                            str(b))

            do_block(blocks[order[0]], filler=fill_one, drain_at=99)
            while fillq:
                fillq.pop(0)()
            emit_units(16)
            prev = order[0]
            rest = list(order[1:])
            pj = [None]
            if rest:
                b = rest.pop(0)
                pjx = mkproj(prev)
                do_block(blocks[b], filler=lambda: pjx.step(2))
                pjx.finish()
                prev = b
            pj[0] = mkproj(prev)
            # ---- local windowed attention ----
            slocs = {}
            ets = {}
            for k2 in range(2):
                sps = ps3.tile([128, 2, 512], f32, tag="sT", bufs=2,
                               name=f"slocps{k2}")
                for hh in range(2):
                    for u in range(8):
                        nc.tensor.matmul(
                            sps[:, hh, 0:WIN],
                            lhsT=KP[:, hh, u, k2 * 8:(k2 + 1) * 8, :],
                            rhs=QP[:, hh, u, :, :],
                            start=(u == 0), stop=(u == 7),
                            skip_group_check=True)
                slocs[k2] = sps
                pj[0].step(2)
            for k2 in range(2):
                et = sb3.tile([128, 2, WIN], bf, tag="eloc", bufs=2,
                              name=f"eloc{k2}")
                nc.scalar.activation(et, slocs[k2][:, :, 0:WIN], AF.Exp,
                                     scale=SCALE / C)
                for hh in range(2):
                    nc.vector.tensor_mul(et[:, hh, :], et[:, hh, :],
                                         lmask_sb[:, k2, :])
                    ets[(k2, hh)] = et[:, hh, :]
            for hh in range(2):
                ploc = ps3.tile([65, WIN], f32, tag="aux", bufs=2,
                                name=f"plocps{hh}")
                for k2 in range(2):
                    nc.tensor.matmul(ploc, lhsT=v256_sb[:, hh, k2, :],
                                     rhs=ets[(k2, hh)], start=(k2 == 0),
                                     stop=False, skip_group_check=True)
                nc.tensor.matmul(ploc, lhsT=tailv_sb[:, hh, :],
                                 rhs=onesrow, start=False, stop=True,
                                 skip_group_check=True)
                zsl = sb3.tile([1, WIN], f32, tag="zsl", bufs=2)
                nc.vector.tensor_copy(zsl, ploc[64:65, :])
                zl = sb3.tile([1, WIN], f32, tag="zl", bufs=2)
                nc.vector.reciprocal_approx_fast(zl, zsl)
                rbls = sb3.tile([64, WIN], f32, tag="rbls", bufs=2)
                nc.gpsimd.partition_broadcast(rbls, zl)
                nc.vector.tensor_mul(blocb[hh * 64:(hh + 1) * 64, :],
                                     ploc[0:64, :], rbls)
            pj[0].step(2)
            emit_proj(0, n1p, 2, blocb, "loc")
            for b in rest:
                pjc = pj[0]
                do_block(blocks[b], filler=lambda: pjc.step(3))
                pjc.finish()
                prev = b
                pj[0] = mkproj(prev)
            pj[0].finish()
    nc.compile()
    return nc


def prep_inputs(x, global_attention_mask, W_local_query, W_local_key,
                W_local_value, W_query, W_key, W_value, W_proj, plan=None):
    """Host-side sharding/layout prep. Returns (plan, per-core input dicts,
    const_ctx_row[E])."""
    if plan is None:
        plan = make_plan(global_attention_mask)

    def b(a):
        return np.ascontiguousarray(np.asarray(a, np.float32)).astype(BF)

    x2 = np.asarray(x, np.float32).reshape(S, E)
    xT_np = np.ascontiguousarray(
        x2.T.reshape(8, 128, S).transpose(1, 0, 2)).astype(BF)
    xg = x2[plan['gpad']]
    xgT_np = np.ascontiguousarray(
        xg.T.reshape(8, 128, plan['n1p']).transpose(1, 0, 2)).astype(BF)

    Wq = np.asarray(W_query, np.float32)
    Wk = np.asarray(W_key, np.float32)
    Wv = np.asarray(W_value, np.float32)
    Wp = np.asarray(W_proj, np.float32)
    Wlv = np.asarray(W_local_value, np.float32)

    # local unit weights: per-i interleave [q_i | k_i], [p, i, c, v]
    Wlq = np.asarray(W_local_query, np.float32).reshape(E, 16, 64)
    Wlk = np.asarray(W_local_key, np.float32).reshape(E, 16, 64)
    wlqk_e = np.concatenate([Wlq, Wlk], axis=2)               # [E, 16, 128]
    wlqk_np = np.ascontiguousarray(
        wlqk_e.reshape(8, 128, 16, 128).transpose(1, 2, 0, 3)).astype(F8)
    wt = np.arange(WIN)
    w_of = (wt % 16) * 16 + wt // 16
    lmask_np = np.ascontiguousarray(
        (w_of.reshape(2, 128)[:, :, None] <= w_of[None, None, :])
        .astype(np.float32).transpose(1, 0, 2)).astype(BF)    # [128, 2, WIN]

    # host-computed local-value summaries
    rows16 = (np.arange(H)[:, None] * 128 + np.arange(16)[None, :]).ravel()
    vl16 = (x2[rows16] @ Wlv).reshape(H, 16, E)
    xsumA = x2.reshape(H, 128, E).sum(axis=1)
    colsumA = xsumA @ Wlv
    colsum16 = vl16.sum(axis=1)
    vbarH = colsumA.reshape(H, 16, 64).sum(axis=1)            # [H, 64]
    tailH = (colsumA - colsum16).reshape(H, 16, 64).sum(axis=1)
    const_row = (vbarH.reshape(E) / S) @ Wp                   # [E]

    # causal cutoff columns for masked tiles
    NT = plan['nt']
    cuts_np = np.zeros((128, NT), np.float32)
    for bidx in plan['order']:
        bl = plan['blocks'][bidx]
        for ti in bl['tiles']:
            if ti['nm']:
                cuts_np[:, ti['mcol']] = ti['cut'] - 0.5

    in_maps = []
    for i in range(NCORES):
        cs = slice(i * 128, (i + 1) * 128)
        wqkv_np = np.stack([
            np.ascontiguousarray(
                W[:, cs].reshape(8, 128, 128).transpose(1, 0, 2))
            for W in (Wq, Wk, Wv)], axis=1).astype(BF)        # [128, 3, 8, 128]
        xTl_np = np.ascontiguousarray(
            x2.T[:, i * 256:(i + 1) * 256]
            .reshape(8, 128, 256).transpose(1, 0, 2)).astype(F8)
        v256_np = np.zeros((128, 2, 2, 65), np.float32)
        tail_np = np.zeros((1, 2, 65), np.float32)
        for hh in range(2):
            hg = 2 * i + hh
            for k2 in range(2):
                wt_ = k2 * 128 + np.arange(128)
                k_true = 16 * (wt_ % 16) + wt_ // 16
                r = k_true // 16
                cpos = k_true % 16
                v256_np[:, hh, k2, 0:64] = vl16[
                    hg, r][np.arange(128)[:, None],
                           (cpos * 64)[:, None] + np.arange(64)[None, :]]
            v256_np[:, hh, :, 64] = 1.0
            tail_np[0, hh, 0:64] = tailH[hg]
            tail_np[0, hh, 64] = S - WIN
        in_maps.append({
            "xT": xT_np,
            "xgT": xgT_np,
            "wqkv": wqkv_np,
            "xTl": xTl_np,
            "wlqk": wlqk_np,
            "lmask": lmask_np,
            "wpr": b(Wp[cs, :]),
            "v256": v256_np.astype(BF),
            "tailv": tail_np.astype(BF),
            "cuts": cuts_np,
        })
    return plan, in_maps, const_row


def assemble(plan, partials, const_row, b_proj, global_attention_mask):
    m = np.asarray(global_attention_mask, np.int64).reshape(S)
    bp = np.asarray(b_proj, np.float32)
    acc = np.zeros((plan['nrows'], E), np.float32)
    for r in partials:
        acc += np.asarray(r["outp"], np.float32)
    out = np.zeros((S, E), np.float32)
    # gathered rows: block b's cols [off, off+w) live at outp rows
    # [roff, roff+w)
    gpad = plan['gpad']
    n1 = plan['n1']
    grows = np.empty(plan['n1p'], np.int64)
    for bidx in plan['order']:
        bl = plan['blocks'][bidx]
        grows[bl['off']:bl['off'] + bl['w']] = np.arange(
            bl['roff'], bl['roff'] + bl['w'])
    out[gpad[:n1]] = acc[grows[:n1]]
    m0 = np.where(m == 0)[0]
    out[m0[m0 >= WIN]] = const_row
    loc_rows = m0[m0 < WIN]
    wperm = (loc_rows % 16) * 16 + loc_rows // 16
    out[loc_rows] = acc[plan['n1p'] + wperm]
    out += bp[None, :]
    return out


def kernel(x, global_attention_mask, W_local_query, W_local_key, W_local_value,
           W_query, W_key, W_value, W_proj, b_proj):
    from concourse.bass_utils import run_bass_kernel_spmd

    plan = make_plan(global_attention_mask)
    key = plan_key(plan)
    if key not in _prog_cache:
        _prog_cache[key] = build_program(plan)
    nc = _prog_cache[key]

    plan, in_maps, const_row = prep_inputs(
        x, global_attention_mask, W_local_query, W_local_key, W_local_value,
        W_query, W_key, W_value, W_proj, plan=plan)
    res = run_bass_kernel_spmd(nc, in_maps, core_ids=list(range(NCORES)))
    out = assemble(plan, res.results, const_row, b_proj,
                   global_attention_mask)
    return out[None].astype(np.float32)


# revision 35
# speedup vs baseline: 1.0467x; 1.0092x over previous
"""Trainium2 Bass kernel v13 for nn_MultiHeadAttention_53017076301867.

Strategy (8 cores, tensor-parallel over H=16 heads, 2 heads/core):
  - ctx = mask ? global_attn : local_attn per row. The device computes the
    global branch ONLY for the gathered (sorted) mask==1 query positions,
    in blocks of <=512 gathered columns; causal masking of gathered
    queries vs key tiles uses per-key-partition cutoff columns applied
    with one DVE scalar_tensor_tensor (is_ge x mult) per head, bounded to
    the boundary region [ca, ce).
  - Phase 1 warms the PE clock gate with junk matmuls, then runs the
    gathered-Q and K projections jointly, ec-outer, so accumulation
    tracks per-ec DMA chunk arrival (x^T and gathered-x^T are split into
    per-2ec chunk pairs across both HWDGE queues).
  - The V projection, PE-transposes into V2e, and the fp8 local q/k unit
    projections are emitted as tensor *filler* inside the first (largest)
    attention block's score/exp window, with all of the block's et tiles
    buffered (bufs=17) and its AV accumulation drained in one burst after
    V2e completes. ACT exp therefore starts as soon as K/Q are done.
  - Local windowed branch (q<WIN rows, permuted w~=i*16+j order) as in
    v2 minus the mask-blend: rows are projected directly and the host
    picks mask==0 & q<WIN rows; mask==0 & q>=WIN rows are one constant
    row computed on the host.
  - Output projections of each block run as tensor filler inside the
    next block (Proj.step), casts alternate DVE/ACT, and each block's
    rows leave in one batched DMA. Host sums 8 bf16 partials, scatters
    rows, and adds b_proj.
"""

import numpy as np
import ml_dtypes

S, E, H, WIN, D = 2048, 1024, 16, 256, 64
C = S // WIN            # 8 chunks
NCORES = 8
SCALE = 1.0 / (D ** 0.5)  # 0.125
BF = ml_dtypes.bfloat16
F8 = ml_dtypes.float8_e4m3fn

_prog_cache = {}
FLAGS = dict(mask=True, iota=True, warm=True, batched_out=True)


def make_plan(mask):
    m = np.asarray(mask, np.int64).reshape(S)
    gidx = np.where(m == 1)[0]
    n1 = len(gidx)
    widths = []
    r = n1
    while r > 512:
        widths.append(512)
        r -= 512
    widths.append(max(128, -(-r // 128) * 128))
    n1p = sum(widths)
    gpad = np.concatenate([gidx, np.full(n1p - n1, gidx[-1], np.int64)])
    blocks = []
    off = 0
    for w in widths:
        pb = gpad[off:off + w]
        kt = int(pb.max()) // 128 + 1
        tiles = []
        for t in range(kt):
            c_t = int(np.searchsorted(pb, t * 128))
            cut = np.searchsorted(pb, t * 128 + np.arange(128)).astype(np.int64)
            nm = bool((cut > c_t).any())
            ce = min((int(cut.max()) + 3) & ~3, w)
            tiles.append(dict(t=t, ca=c_t & ~3, ce=ce, cut=cut, nm=nm,
                              mcol=-1))
        wr = w if off + w <= n1 else max(4, (min(w, n1 - off) + 3) & ~3)
        blocks.append(dict(off=off, w=w, kt=kt, tiles=tiles, wr=wr))
        off += w
    order = sorted(range(len(blocks)),
                   key=lambda b: (-blocks[b]['w'], -blocks[b]['kt']))
    nt = 0
    roff = 0
    for b in order:
        bl = blocks[b]
        bl['roff'] = roff
        roff += bl['w']
        for ti in bl['tiles']:
            if ti['nm']:
                ti['mcol'] = nt
                nt += 1
    qgw = []
    r = n1p
    while r > 0:
        qgw.append(min(512, r))
        r -= qgw[-1]
    return dict(n1=n1, n1p=n1p, widths=widths, gpad=gpad, blocks=blocks,
                order=order, nt=max(nt, 1), qgw=qgw, nrows=n1p + WIN)


def plan_key(plan):
    parts = [tuple(plan['widths'])]
    for b in plan['order']:
        bl = plan['blocks'][b]
        parts.append((bl['off'], bl['w'], bl['kt'], bl['roff'], bl['wr'],
                      tuple((ti['t'], ti['ca'], ti['nm'], ti['mcol'])
                            for ti in bl['tiles'])))
    return tuple(parts)


def build_program(plan):
    from contextlib import ExitStack
    import concourse.tile as tile
    import concourse.mybir as mybir
    from concourse import bacc
    from concourse.masks import make_identity

    dt = mybir.dt
    f32, bf, f8 = dt.float32, dt.bfloat16, dt.float8e4
    AF = mybir.ActivationFunctionType
    ALU = mybir.AluOpType

    n1p = plan['n1p']
    NT = plan['nt']
    NR = plan['nrows']

    nc = bacc.Bacc("TRN2", target_bir_lowering=False, debug=False)

    xT = nc.dram_tensor("xT", [128, 8, S], bf, kind="ExternalInput").ap()
    xgT = nc.dram_tensor("xgT", [128, 8, n1p], bf, kind="ExternalInput").ap()
    wqkv = nc.dram_tensor("wqkv", [128, 3, 8, 128], bf, kind="ExternalInput").ap()
    xTl = nc.dram_tensor("xTl", [128, 8, 2 * 128], f8, kind="ExternalInput").ap()
    wlqk = nc.dram_tensor("wlqk", [128, 16, 8, 128], f8, kind="ExternalInput").ap()
    lmask = nc.dram_tensor("lmask", [128, 2, WIN], bf, kind="ExternalInput").ap()
    wpr = nc.dram_tensor("wpr", [128, E], bf, kind="ExternalInput").ap()
    v256 = nc.dram_tensor("v256", [128, 2, 2, 65], bf, kind="ExternalInput").ap()
    tailv = nc.dram_tensor("tailv", [1, 2, 65], bf, kind="ExternalInput").ap()
    cuts = nc.dram_tensor("cuts", [128, NT], f32, kind="ExternalInput").ap()
    outp = nc.dram_tensor("outp", [NR, E], bf, kind="ExternalOutput").ap()
    outv = outp.rearrange("(r p) e -> p r e", p=128)

    with tile.TileContext(nc) as tc, ExitStack() as ctx:
        P = ctx.enter_context(tc.tile_pool(name="persist", bufs=1))

        # ---------------- input DMAs (order = per-queue priority) --------
        xgT_sb = P.tile([128, 8, n1p], bf)
        xT_sb = P.tile([128, 8, S], bf)
        wqkv_sb = P.tile([128, 3, 8, 128], bf)
        xTl_sb = P.tile([128, 8, 256], f8)
        wlqk_sb = P.tile([128, 16, 8, 128], f8)
        # per-ec (xgT, xT) pairs split across the two HWDGE queues so the
        # joint ec-outer Q/K accumulation can start as soon as each ec lands
        nc.sync.dma_start(out=wqkv_sb, in_=wqkv)
        nc.scalar.dma_start(out=xTl_sb, in_=xTl)
        for c2 in range(4):
            eng = nc.sync if c2 % 2 == 0 else nc.scalar
            sl = slice(2 * c2, 2 * c2 + 2)
            eng.dma_start(out=xgT_sb[:, sl, :], in_=xgT[:, sl, :])
            eng.dma_start(out=xT_sb[:, sl, :], in_=xT[:, sl, :])
        for q4 in range(4):
            eng = nc.sync if q4 % 2 == 0 else nc.scalar
            eng.dma_start(out=wlqk_sb[:, 4 * q4:4 * q4 + 4, :, :],
                          in_=wlqk[:, 4 * q4:4 * q4 + 4, :, :])
        lmask_sb = P.tile([128, 2, WIN], bf)
        nc.sync.dma_start(out=lmask_sb, in_=lmask)
        v256_sb = P.tile([128, 2, 2, 65], bf)
        nc.scalar.dma_start(out=v256_sb, in_=v256)
        tailv_sb = P.tile([1, 2, 65], bf)
        nc.sync.dma_start(out=tailv_sb, in_=tailv)
        wpr_sb = P.tile([128, E], bf)
        nc.scalar.dma_start(out=wpr_sb, in_=wpr)
        cuts_sb = P.tile([128, NT], f32)
        nc.gpsimd.dma_start(out=cuts_sb, in_=cuts)

        # ---------------- setup ----------------
        warm = P.tile([128, 512], bf)
        nc.vector.memset(warm, 0.125)
        onesrow = P.tile([1, WIN], bf)
        nc.vector.memset(onesrow, 1.0)
        identb128 = P.tile([128, 128], bf)
        make_identity(nc, identb128)
        colidx = P.tile([128, 512], f32)
        nc.gpsimd.iota(colidx, pattern=[[1, 512]], base=0,
                       channel_multiplier=0,
                       allow_small_or_imprecise_dtypes=True)

        QT2g = P.tile([128, n1p], bf)
        KT2 = P.tile([128, S], bf)
        V2e = P.tile([128, 16, 2, 65], bf)
        QP = P.tile([64, 2, 8, 16, 16], bf)
        KP = P.tile([64, 2, 8, 16, 16], bf)
        ctxT = P.tile([128, n1p], bf)
        nc.gpsimd.memset(ctxT, 0.0)
        blocb = P.tile([128, WIN], bf)

        # ------- phase 1a: warm-up, joint ec-outer gathered-Q + K -------
        with tc.tile_pool(name="ps1", bufs=1, space="PSUM") as ps1:
            wps = ps1.tile([128, 512], f32, tag="warm", bufs=1,
                           name="warmps")
            for _ in range(16):
                nc.tensor.matmul(wps, lhsT=warm[:, 0:128], rhs=warm,
                                 start=True, stop=True, skip_group_check=True)
            qgw = plan['qgw']
            qps = [ps1.tile([128, 512], f32, tag=f"qg{g}", bufs=1,
                            name=f"qgps{g}") for g in range(len(qgw))]
            kp0 = ps1.tile([128, 512], f32, tag="kv0", bufs=1, name="kps0")
            uidx = [0]

            for ec in range(8):
                qoff = 0
                for g, gw in enumerate(qgw):
                    nc.tensor.matmul(qps[g][:, 0:gw],
                                     lhsT=wqkv_sb[:, 0, ec, :],
                                     rhs=xgT_sb[:, ec, qoff:qoff + gw],
                                     start=(ec == 0), stop=(ec == 7),
                                     skip_group_check=True)
                    qoff += gw
                nc.tensor.matmul(
                    kp0, lhsT=wqkv_sb[:, 1, ec, :],
                    rhs=xT_sb[:, ec, 0:512],
                    start=(ec == 0), stop=(ec == 7),
                    skip_group_check=True)
                # junk matmuls bridge DMA-arrival jitter (keep HAM at 8/8)
                for _ in range(2):
                    nc.tensor.matmul(wps, lhsT=warm[:, 0:128], rhs=warm,
                                     start=True, stop=True,
                                     skip_group_check=True)
            qoff = 0
            for g, gw in enumerate(qgw):
                nc.vector.tensor_copy(QT2g[:, qoff:qoff + gw],
                                      qps[g][:, 0:gw])
                qoff += gw
            nc.vector.tensor_copy(KT2[:, 0:512], kp0)

        # ------- phase 2: V, attention blocks, local, projections -------
        with tc.tile_pool(name="ps3", bufs=2, space="PSUM") as ps3, \
                tc.tile_pool(name="sb3", bufs=4) as sb3:
            # V projection + transpose into V2e — emitted as filler work
            # inside the first attention block's score/exp window
            VT2 = sb3.tile([128, S], bf, tag="vt2", bufs=1)

            def emit_vgroup(g):
                ps = ps3.tile([128, 512], f32, tag="aux", bufs=2, name="vps")
                for ec in range(8):
                    nc.tensor.matmul(
                        ps, lhsT=wqkv_sb[:, 2, ec, :],
                        rhs=xT_sb[:, ec, g * 512:(g + 1) * 512],
                        start=(ec == 0), stop=(ec == 7))
                nc.vector.tensor_copy(VT2[:, g * 512:(g + 1) * 512], ps)

            def emit_vtr(st):
                pv = ps3.tile([128, 128], bf, tag="aux", bufs=2, name="pvps")
                nc.tensor.transpose(pv, VT2[:, st * 128:(st + 1) * 128],
                                    identb128)
                nc.vector.tensor_copy(V2e[:, st, :, 0:64],
                                      pv.rearrange("p (h d) -> p h d", h=2))
                if st == 15:
                    nc.gpsimd.memset(V2e[:, :, :, 64], 1.0)

            def emit_kgroup(g):
                ps = ps3.tile([128, 512], f32, tag="aux", bufs=2, name="kps")
                for ec in range(8):
                    nc.tensor.matmul(
                        ps, lhsT=wqkv_sb[:, 1, ec, :],
                        rhs=xT_sb[:, ec, g * 512:(g + 1) * 512],
                        start=(ec == 0), stop=(ec == 7))
                nc.vector.tensor_copy(KT2[:, g * 512:(g + 1) * 512], ps)

            fillq = []
            for g in range(1, 4):
                fillq.append(lambda g=g: emit_kgroup(g))
            for g in range(4):
                fillq.append(lambda g=g: emit_vgroup(g))
            for s4 in range(4):
                def vtr4(s4=s4):
                    for st in range(4 * s4, 4 * s4 + 4):
                        emit_vtr(st)
                fillq.append(vtr4)

            def fill_one():
                if fillq:
                    fillq.pop(0)()
                else:
                    emit_units(1)

            # local q/k units, emitted as filler inside the first block

            wlqk_dr = wlqk_sb.rearrange("p i (g j) m -> p i g j m", j=2)
            xTl_dr = xTl_sb.rearrange("p (g j) s -> p g j s", j=2)

            def emit_units(n):
                while uidx[0] < 16 and n > 0:
                    i = uidx[0]
                    uidx[0] += 1
                    n -= 1
                    ps = ps3.tile([128, 256], f32, tag="aux", bufs=2,
                                  name="ups")
                    for ec in range(8):
                        nc.tensor.matmul(ps, lhsT=wlqk_sb[:, i, ec, :],
                                         rhs=xTl_sb[:, ec, :],
                                         start=(ec == 0), stop=(ec == 7))
                    qsrc = ps[0:64, :].rearrange("d (h u j) -> d h u j",
                                                 h=2, u=8)
                    ksrc = ps[64:128, :].rearrange("d (h u j) -> d h u j",
                                                   h=2, u=8)
                    nc.vector.tensor_copy(QP[:, :, :, i, :], qsrc)
                    nc.vector.tensor_copy(KP[:, :, :, i, :], ksrc)

            mcnt = [0]

            def emit_mask(et, sl_et, iosl, cutcol):
                mcnt[0] += 1
                nc.vector.scalar_tensor_tensor(
                    out=et[sl_et], in0=colidx[:, iosl], scalar=cutcol,
                    in1=et[sl_et], op0=ALU.is_ge, op1=ALU.mult)

            def do_block(bl, filler=None, drain_at=2, etbufs=17,
                         drain_gate=None):
                w, off, kt, wr = bl['w'], bl['off'], bl['kt'], bl['wr']
                pack = 512 // w
                tiles = bl['tiles']
                ngrp = -(-len(tiles) // pack)
                gpss = ps3.tile([65, 2, 512], f32, tag="g01", bufs=1,
                                name="gctxps")
                pend = []
                for gi in range(2 * ngrp + 2):
                    if gi < ngrp:
                        grp = tiles[gi * pack:(gi + 1) * pack]
                        ca0 = grp[0]['ca'] if pack == 1 else 0
                        sps = ps3.tile([128, 2, 512], f32, tag="sT", bufs=2,
                                       name="sTps")
                        for qi, ti in enumerate(grp):
                            t, ca = ti['t'], ti['ca']
                            for hh in range(2):
                                hs = slice(hh * 64, hh * 64 + 64)
                                nc.tensor.matmul(
                                    sps[:, hh, qi * w + ca:qi * w + wr],
                                    lhsT=KT2[hs, t * 128:(t + 1) * 128],
                                    rhs=QT2g[hs, off + ca:off + wr],
                                    start=(qi == 0), stop=(qi == len(grp) - 1),
                                    skip_group_check=True)
                        et = sb3.tile([128, 2, 512], bf, tag="expT",
                                      bufs=etbufs, name="etT")
                        if pack == 1:
                            nc.scalar.activation(et[:, :, ca0:wr],
                                                 sps[:, :, ca0:wr], AF.Exp,
                                                 scale=SCALE)
                        else:
                            etv = et.rearrange("p h (g q) -> p h g q", q=w)
                            spsv = sps.rearrange("p h (g q) -> p h g q", q=w)
                            nc.scalar.activation(
                                etv[:, :, 0:len(grp), ca0:wr],
                                spsv[:, :, 0:len(grp), ca0:wr], AF.Exp,
                                scale=SCALE)
                        for qi, ti in enumerate(grp):
                            if ti['nm'] and FLAGS['mask']:
                                ca = ti['ca']
                                ce = min(ti['ce'], wr)
                                cutcol = cuts_sb[:, ti['mcol']:ti['mcol'] + 1]
                                for hh in range(2):
                                    emit_mask(
                                        et, (slice(None), hh,
                                             slice(qi * w + ca, qi * w + ce)),
                                        slice(ca, ce), cutcol)
                        pend.append((grp, et))
                        if filler is not None:
                            filler()
                    ready = drain_gate is None or drain_gate()
                    if (ready and len(pend) > drain_at) or (gi >= ngrp
                                                           and pend):
                        pgrp, pet = pend.pop(0)
                        for qi, ti in enumerate(pgrp):
                            t, ca = ti['t'], ti['ca']
                            for hh in range(2):
                                nc.tensor.matmul(
                                    gpss[:, hh, ca:wr],
                                    lhsT=V2e[:, t, hh, :],
                                    rhs=pet[:, hh, qi * w + ca:qi * w + wr],
                                    start=(t == 0), stop=(t == kt - 1),
                                    skip_group_check=True)
                # blend: ctxT = gpss[0:64] / gpss[64]
                zsr = sb3.tile([1, 2, wr], f32, tag=f"zsr{w}", bufs=2)
                nc.vector.tensor_copy(zsr, gpss[64:65, :, 0:wr])
                zrow = sb3.tile([1, 2, wr], f32, tag=f"zrow{w}", bufs=2)
                nc.vector.reciprocal_approx_fast(zrow, zsr)
                rbs = sb3.tile([64, 2, wr], f32, tag=f"rbs{w}", bufs=2)
                nc.gpsimd.partition_broadcast(rbs, zrow)
                for hh in range(2):
                    hs = slice(hh * 64, hh * 64 + 64)
                    nc.vector.tensor_mul(ctxT[hs, off:off + wr],
                                         gpss[0:64, hh, 0:wr], rbs[:, hh, :])

            pcnt = [0]

            class Proj:
                def __init__(self, colbase, roff, nqt, src, bname):
                    self.colbase, self.roff, self.nqt = colbase, roff, nqt
                    self.src, self.bname = src, bname
                    self.idx = 0
                    self.stg = sb3.tile([128, nqt, E], bf,
                                        tag=f"stg{bname}", bufs=1,
                                        name=f"stg{bname}")

                def step(self, n=1):
                    while self.idx < 2 * self.nqt and n > 0:
                        j, half = divmod(self.idx, 2)
                        self.idx += 1
                        n -= 1
                        pp = ps3.tile([128, 512], f32, tag="aux", bufs=2,
                                      name="ppps")
                        nc.tensor.matmul(
                            pp, lhsT=self.src[:, self.colbase + j * 128:
                                              self.colbase + (j + 1) * 128],
                            rhs=wpr_sb[:, half * 512:(half + 1) * 512],
                            start=True, stop=True)
                        dst = self.stg[:, j, half * 512:(half + 1) * 512]
                        pcnt[0] += 1
                        if pcnt[0] % 2:
                            nc.vector.tensor_copy(dst, pp)
                        else:
                            nc.scalar.copy(dst, pp)

                def finish(self):
                    self.step(2 * self.nqt)
                    if FLAGS['batched_out']:
                        nc.sync.dma_start(
                            out=outv[:, self.roff // 128:
                                     self.roff // 128 + self.nqt, :],
                            in_=self.stg)
                    else:
                        for j in range(self.nqt):
                            nc.sync.dma_start(
                                out=outp[self.roff + j * 128:
                                         self.roff + (j + 1) * 128, :],
                                in_=self.stg[:, j, :])

            def emit_proj(colbase, roff, nqt, src, bname, cast_eng=None):
                Proj(colbase, roff, nqt, src, bname).finish()

            # ---- blocks in order; local chain after the second block ----
            order = plan['order']
            blocks = plan['blocks']

            def mkproj(b):
                bl = blocks[b]
                return Proj(bl['off'], bl['roff'], bl['w'] // 128, ctxT,
                            str(b))

            do_block(blocks[order[0]], filler=fill_one, drain_at=99)
            while fillq:
                fillq.pop(0)()
            emit_units(16)
            prev = order[0]
            rest = list(order[1:])
            pj = [None]
            if rest:
                b = rest.pop(0)
                pjx = mkproj(prev)
                do_block(blocks[b], filler=lambda: pjx.step(2))
                pjx.finish()
                prev = b
            pj[0] = mkproj(prev)
            # ---- local windowed attention ----
            slocs = {}
            ets = {}
            for k2 in range(2):
                sps = ps3.tile([128, 2, 512], f32, tag="sT", bufs=2,
                               name=f"slocps{k2}")
                for hh in range(2):
                    for u in range(8):
                        nc.tensor.matmul(
                            sps[:, hh, 0:WIN],
                            lhsT=KP[:, hh, u, k2 * 8:(k2 + 1) * 8, :],
                            rhs=QP[:, hh, u, :, :],
                            start=(u == 0), stop=(u == 7),
                            skip_group_check=True)
                slocs[k2] = sps
                pj[0].step(2)
            for k2 in range(2):
                et = sb3.tile([128, 2, WIN], bf, tag="eloc", bufs=2,
                              name=f"eloc{k2}")
                nc.scalar.activation(et, slocs[k2][:, :, 0:WIN], AF.Exp,
                                     scale=SCALE / C)
                for hh in range(2):
                    nc.vector.tensor_mul(et[:, hh, :], et[:, hh, :],
                                         lmask_sb[:, k2, :])
                    ets[(k2, hh)] = et[:, hh, :]
            for hh in range(2):
                ploc = ps3.tile([65, WIN], f32, tag="aux", bufs=2,
                                name=f"plocps{hh}")
                for k2 in range(2):
                    nc.tensor.matmul(ploc, lhsT=v256_sb[:, hh, k2, :],
                                     rhs=ets[(k2, hh)], start=(k2 == 0),
                                     stop=False, skip_group_check=True)
                nc.tensor.matmul(ploc, lhsT=tailv_sb[:, hh, :],
                                 rhs=onesrow, start=False, stop=True,
                                 skip_group_check=True)
                zsl = sb3.tile([1, WIN], f32, tag="zsl", bufs=2)
                nc.vector.tensor_copy(zsl, ploc[64:65, :])
                zl = sb3.tile([1, WIN], f32, tag="zl", bufs=2)
                nc.vector.reciprocal_approx_fast(zl, zsl)
                rbls = sb3.tile([64, WIN], f32, tag="rbls", bufs=2)
                nc.gpsimd.partition_broadcast(rbls, zl)
                nc.vector.tensor_mul(blocb[hh * 64:(hh + 1) * 64, :],
                                     ploc[0:64, :], rbls)
            pj[0].step(2)
            emit_proj(0, n1p, 2, blocb, "loc")
            for b in rest:
                pjc = pj[0]
                do_block(blocks[b], filler=lambda: pjc.step(3))
                pjc.finish()
                prev = b
                pj[0] = mkproj(prev)
            pj[0].finish()
    nc.compile()
    return nc


def prep_inputs(x, global_attention_mask, W_local_query, W_local_key,
                W_local_value, W_query, W_key, W_value, W_proj, plan=None):
    """Host-side sharding/layout prep. Returns (plan, per-core input dicts,
    const_ctx_row[E])."""
    if plan is None:
        plan = make_plan(global_attention_mask)

    def b(a):
        return np.ascontiguousarray(np.asarray(a, np.float32)).astype(BF)

    x2 = np.asarray(x, np.float32).reshape(S, E)
    xT_np = np.ascontiguousarray(
        x2.T.reshape(8, 128, S).transpose(1, 0, 2)).astype(BF)
    xg = x2[plan['gpad']]
    xgT_np = np.ascontiguousarray(
        xg.T.reshape(8, 128, plan['n1p']).transpose(1, 0, 2)).astype(BF)

    Wq = np.asarray(W_query, np.float32)
    Wk = np.asarray(W_key, np.float32)
    Wv = np.asarray(W_value, np.float32)
    Wp = np.asarray(W_proj, np.float32)
    Wlv = np.asarray(W_local_value, np.float32)

    # local unit weights: per-i interleave [q_i | k_i], [p, i, c, v]
    Wlq = np.asarray(W_local_query, np.float32).reshape(E, 16, 64)
    Wlk = np.asarray(W_local_key, np.float32).reshape(E, 16, 64)
    wlqk_e = np.concatenate([Wlq, Wlk], axis=2)               # [E, 16, 128]
    wlqk_np = np.ascontiguousarray(
        wlqk_e.reshape(8, 128, 16, 128).transpose(1, 2, 0, 3)).astype(F8)
    wt = np.arange(WIN)
    w_of = (wt % 16) * 16 + wt // 16
    lmask_np = np.ascontiguousarray(
        (w_of.reshape(2, 128)[:, :, None] <= w_of[None, None, :])
        .astype(np.float32).transpose(1, 0, 2)).astype(BF)    # [128, 2, WIN]

    # host-computed local-value summaries
    rows16 = (np.arange(H)[:, None] * 128 + np.arange(16)[None, :]).ravel()
    vl16 = (x2[rows16] @ Wlv).reshape(H, 16, E)
    xsumA = x2.reshape(H, 128, E).sum(axis=1)
    colsumA = xsumA @ Wlv
    colsum16 = vl16.sum(axis=1)
    vbarH = colsumA.reshape(H, 16, 64).sum(axis=1)            # [H, 64]
    tailH = (colsumA - colsum16).reshape(H, 16, 64).sum(axis=1)
    const_row = (vbarH.reshape(E) / S) @ Wp                   # [E]

    # causal cutoff columns for masked tiles
    NT = plan['nt']
    cuts_np = np.zeros((128, NT), np.float32)
    for bidx in plan['order']:
        bl = plan['blocks'][bidx]
        for ti in bl['tiles']:
            if ti['nm']:
                cuts_np[:, ti['mcol']] = ti['cut'] - 0.5

    in_maps = []
    for i in range(NCORES):
        cs = slice(i * 128, (i + 1) * 128)
        wqkv_np = np.stack([
            np.ascontiguousarray(
                W[:, cs].reshape(8, 128, 128).transpose(1, 0, 2))
            for W in (Wq, Wk, Wv)], axis=1).astype(BF)        # [128, 3, 8, 128]
        xTl_np = np.ascontiguousarray(
            x2.T[:, i * 256:(i + 1) * 256]
            .reshape(8, 128, 256).transpose(1, 0, 2)).astype(F8)
        v256_np = np.zeros((128, 2, 2, 65), np.float32)
        tail_np = np.zeros((1, 2, 65), np.float32)
        for hh in range(2):
            hg = 2 * i + hh
            for k2 in range(2):
                wt_ = k2 * 128 + np.arange(128)
                k_true = 16 * (wt_ % 16) + wt_ // 16
                r = k_true // 16
                cpos = k_true % 16
                v256_np[:, hh, k2, 0:64] = vl16[
                    hg, r][np.arange(128)[:, None],
                           (cpos * 64)[:, None] + np.arange(64)[None, :]]
            v256_np[:, hh, :, 64] = 1.0
            tail_np[0, hh, 0:64] = tailH[hg]
            tail_np[0, hh, 64] = S - WIN
        in_maps.append({
            "xT": xT_np,
            "xgT": xgT_np,
            "wqkv": wqkv_np,
            "xTl": xTl_np,
            "wlqk": wlqk_np,
            "lmask": lmask_np,
            "wpr": b(Wp[cs, :]),
            "v256": v256_np.astype(BF),
            "tailv": tail_np.astype(BF),
            "cuts": cuts_np,
        })
    return plan, in_maps, const_row


def assemble(plan, partials, const_row, b_proj, global_attention_mask):
    m = np.asarray(global_attention_mask, np.int64).reshape(S)
    bp = np.asarray(b_proj, np.float32)
    acc = np.zeros((plan['nrows'], E), np.float32)
    for r in partials:
        acc += np.asarray(r["outp"], np.float32)
    out = np.zeros((S, E), np.float32)
    # gathered rows: block b's cols [off, off+w) live at outp rows
    # [roff, roff+w)
    gpad = plan['gpad']
    n1 = plan['n1']
    grows = np.empty(plan['n1p'], np.int64)
    for bidx in plan['order']:
        bl = plan['blocks'][bidx]
        grows[bl['off']:bl['off'] + bl['w']] = np.arange(
            bl['roff'], bl['roff'] + bl['w'])
    out[gpad[:n1]] = acc[grows[:n1]]
    m0 = np.where(m == 0)[0]
    out[m0[m0 >= WIN]] = const_row
    loc_rows = m0[m0 < WIN]
    wperm = (loc_rows % 16) * 16 + loc_rows // 16
    out[loc_rows] = acc[plan['n1p'] + wperm]
    out += bp[None, :]
    return out


def kernel(x, global_attention_mask, W_local_query, W_local_key, W_local_value,
           W_query, W_key, W_value, W_proj, b_proj):
    from concourse.bass_utils import run_bass_kernel_spmd

    plan = make_plan(global_attention_mask)
    key = plan_key(plan)
    if key not in _prog_cache:
        _prog_cache[key] = build_program(plan)
    nc = _prog_cache[key]

    plan, in_maps, const_row = prep_inputs(
        x, global_attention_mask, W_local_query, W_local_key, W_local_value,
        W_query, W_key, W_value, W_proj, plan=plan)
    res = run_bass_kernel_spmd(nc, in_maps, core_ids=list(range(NCORES)))
    out = assemble(plan, res.results, const_row, b_proj,
                   global_attention_mask)
    return out[None].astype(np.float32)
